# revision 9
# baseline (speedup 1.0000x reference)
"""Trainium2 Bass kernel for nn_FFTBias2d (B=16, H=16, E=64, s=48).

Math: the reference's FFT pipeline collapses exactly. With
  z = exp(zsym - offset) = exp(-offset[b,h]) * exp(zsym[h]),
the per-(b,h) frequency response factors into a scalar exp(-offset) times a
per-head constant, so
  RxV = scale[b,h] * Mbase[h] @ colsum,   RxU = scale[b,h] * Mbase[h] @ rowsum,
  Mbase[h] = G @ diag_c(rfft95(exp(zsym[h]))) @ A48   (a real 48x48 matrix)
where G = (irfft94(.)[:48]) as a real matrix and A48 = rfft95 of a signal
supported on positions 47..94. rowsum/colsum are the 48-row/col sums of each
48x48 spatial image in v, and the output is the broadcast sum
  pbv[i*48+j] = RxU[i] + RxV[j],  z_pb analog with zv = 48*scale*Mbase@1.

Device pipeline per core (2 batches of the 16, data-parallel over 8 cores):
  phase 0: build Mt[b,h] = (scale*Mbase[h]).T on-chip from zsym/offset inputs
  phase A: 18 accumulating matmuls vs a 0/1 matrix -> rowsum||colsum  [reduce]
  phase B: per-head 48x48 matvecs -> RxU/RxV (+ zv columns)
  phase C: 18 expand matmuls vs a 0/1 matrix -> output tiles -> DMA   [expand]

Engine layout: loads are f32->f16 casting DMAs issued by GpSimd (SWDGE),
stores go out on the ACT HWDGE ring, PSUM evacuation on DVE. Emission order
interleaves batches (A0, ph0, B0, A1, C0, B1, C1) so HBM never idles.
"""

import numpy as np

import concourse.bass as bass
import concourse.mybir as mybir
from concourse import bacc
import concourse.tile as tile
from concourse.bass_utils import run_bass_kernel_spmd

B, SP2, H, E = 16, 2306, 16, 64
S1 = 48                      # spatial side
NPAD = 2 * S1 - 1            # 95
NIRFFT = 2 * (S1 - 1)        # 94
HE = H * E                   # 1024
ROWS = S1 * S1               # 2304
NCHUNK = ROWS // 128         # 18
NCORES = 8
BL = B // NCORES             # 2 batches per core

# dtype knobs for the two big matmul stages.
ST1 = "f16"   # reduce stage (v -> row/col sums)
ST2 = "f16"   # expand stage (RxU/RxV -> output)

TRACE = False
LAST_RESULTS = None

_DT = {"f32": mybir.dt.float32, "f16": mybir.dt.float16, "f32r": mybir.dt.float32}
_NPDT = {"f32": np.float32, "f16": np.float16, "f32r": np.float32}


def _build_host_constants():
    s = S1
    # G: irfft_94(.)[:48] as a real [48, 96] matrix acting on (Re, Im) stacked
    G = np.zeros((s, 2 * s), dtype=np.float64)
    for k in range(s):
        e = np.zeros(s, dtype=np.complex128)
        e[k] = 1.0
        G[:, k] = np.fft.irfft(e, n=NIRFFT)[:s]
        e[k] = 1j
        G[:, s + k] = np.fft.irfft(e, n=NIRFFT)[:s]

    # A48: rfft_95 of impulse at position 47+t  -> [48 bins, 48 positions]
    Are = np.zeros((s, s), dtype=np.float64)
    Aim = np.zeros((s, s), dtype=np.float64)
    for t in range(s):
        e = np.zeros(NPAD, dtype=np.float64)
        e[s - 1 + t] = 1.0
        F = np.fft.rfft(e)
        Are[:, t] = F.real
        Aim[:, t] = F.imag

    # rfft_95 full matrix -> [48, 95]
    FrRe = np.zeros((s, NPAD), dtype=np.float64)
    FrIm = np.zeros((s, NPAD), dtype=np.float64)
    for p in range(NPAD):
        e = np.zeros(NPAD, dtype=np.float64)
        e[p] = 1.0
        F = np.fft.rfft(e)
        FrRe[:, p] = F.real
        FrIm[:, p] = F.imag

    # duplicated-rfft weights: out rows 0:48 and 48:96 both get the same part
    frre_dup = np.zeros((NPAD, 96), dtype=np.float32)
    frre_dup[:, 0:48] = FrRe.T
    frre_dup[:, 48:96] = FrRe.T
    frim_dup = np.zeros((NPAD, 96), dtype=np.float32)
    frim_dup[:, 0:48] = FrIm.T
    frim_dup[:, 48:96] = FrIm.T

    # a2/b2: T = a2 * Zre_dup + b2 * Zim_dup  gives [re_rows; im_rows] of
    # diag_c(Z) @ A48:  re = Zre*Are - Zim*Aim ; im = Zre*Aim + Zim*Are
    a2 = np.zeros((96, 48), dtype=np.float32)
    a2[0:48] = Are
    a2[48:96] = Aim
    b2 = np.zeros((96, 48), dtype=np.float32)
    b2[0:48] = -Aim
    b2[48:96] = Are

    gt = np.ascontiguousarray(G.T.astype(np.float32))  # [96, 48]

    # reduce-stage weight: lhsT [128, 112] per chunk;
    # cols 0:48 pick rowsum (i = p//48), cols 64:112 pick colsum (j = p%48)
    at = np.zeros((128, NCHUNK * 112), dtype=np.float64)
    for c in range(NCHUNK):
        for k in range(128):
            p = c * 128 + k
            at[k, c * 112 + (p // S1)] = 1.0
            at[k, c * 112 + 64 + (p % S1)] = 1.0

    # expand-stage weight: lhsT [128, 128] per chunk;
    # out row m (global p = c*128+m) = rhs[i(p)] + rhs[64 + j(p)]
    ew = np.zeros((128, NCHUNK * 128), dtype=np.float64)
    for c in range(NCHUNK):
        for m in range(128):
            p = c * 128 + m
            ew[p // S1, c * 128 + m] = 1.0
            ew[64 + (p % S1), c * 128 + m] = 1.0

    ones112 = np.ones((1, 112), dtype=np.float32)

    perm = np.concatenate(
        [[S1 - 1], np.arange(S1 - 1, 0, -1), np.arange(0, S1 - 1)]
    )  # z = concat(w[-1:], w[1:][::-1], w[:-1])

    return {
        "frre": frre_dup, "frim": frim_dup, "a2": a2, "b2": b2, "gt": gt,
        "at": at.astype(_NPDT[ST1]), "ew": ew.astype(_NPDT[ST2]),
        "ones112": ones112, "perm": perm,
    }


def _build_nc():
    f32 = mybir.dt.float32
    st1 = _DT[ST1]
    st2 = _DT[ST2]

    nc = bacc.Bacc()

    v_t = nc.declare_dram_parameter("v", [BL, SP2, HE], f32, isOutput=False)
    off_t = nc.declare_dram_parameter("offset", [1, BL * H], f32, isOutput=False)
    zsym_t = nc.declare_dram_parameter("zsymt", [NPAD, H], f32, isOutput=False)
    at_t = nc.declare_dram_parameter("at_c", [128, NCHUNK * 112], st1, isOutput=False)
    ew_t = nc.declare_dram_parameter("ew_c", [128, NCHUNK * 128], st2, isOutput=False)
    frre_t = nc.declare_dram_parameter("frre", [NPAD, 96], f32, isOutput=False)
    frim_t = nc.declare_dram_parameter("frim", [NPAD, 96], f32, isOutput=False)
    a2_t = nc.declare_dram_parameter("a2", [96, 48], f32, isOutput=False)
    b2_t = nc.declare_dram_parameter("b2", [96, 48], f32, isOutput=False)
    gt_t = nc.declare_dram_parameter("gt", [96, 48], f32, isOutput=False)
    ones_t = nc.declare_dram_parameter("ones112", [1, 112], f32, isOutput=False)

    pbv_t = nc.declare_dram_parameter("pbv", [BL, SP2, HE], f32, isOutput=True)
    zpb_t = nc.declare_dram_parameter("zpb", [BL, SP2, H], f32, isOutput=True)

    def mm(out, lhsT, rhs, **kw):
        nc.tensor.matmul(out, lhsT, rhs, **kw)

    with tile.TileContext(nc) as tc:
        from contextlib import ExitStack

        with ExitStack() as ctx:
            consts = ctx.enter_context(tc.tile_pool(name="consts", bufs=1))
            small = ctx.enter_context(tc.tile_pool(name="small", bufs=2))
            vt_pool = ctx.enter_context(tc.tile_pool(name="vt", bufs=8))
            sums_pool = ctx.enter_context(tc.tile_pool(name="sums", bufs=2))
            ob_pool = ctx.enter_context(tc.tile_pool(name="ob", bufs=3))
            ps_acc = ctx.enter_context(tc.tile_pool(name="ps_acc", bufs=2, space="PSUM"))
            ps_out = ctx.enter_context(tc.tile_pool(name="ps_out", bufs=3, space="PSUM"))
            ps_small = ctx.enter_context(tc.tile_pool(name="ps_small", bufs=1, space="PSUM"))

            # ---- constants ----
            at_sb = consts.tile([128, NCHUNK * 112], st1)
            nc.sync.dma_start(out=at_sb, in_=at_t[:])
            ew_sb = consts.tile([128, NCHUNK * 128], st2)
            nc.sync.dma_start(out=ew_sb, in_=ew_t[:])
            frre_sb = consts.tile([NPAD, 96], f32)
            nc.sync.dma_start(out=frre_sb, in_=frre_t[:])
            frim_sb = consts.tile([NPAD, 96], f32)
            nc.sync.dma_start(out=frim_sb, in_=frim_t[:])
            a2_sb = consts.tile([96, 48], f32)
            nc.sync.dma_start(out=a2_sb, in_=a2_t[:])
            b2_sb = consts.tile([96, 48], f32)
            nc.sync.dma_start(out=b2_sb, in_=b2_t[:])
            gt_sb = consts.tile([96, 48], f32)
            nc.sync.dma_start(out=gt_sb, in_=gt_t[:])
            ones_sb = consts.tile([1, 112], f32)
            nc.sync.dma_start(out=ones_sb, in_=ones_t[:])
            zsym_sb = consts.tile([NPAD, H], f32)
            nc.sync.dma_start(out=zsym_sb, in_=zsym_t[:])
            off_sb = consts.tile([1, BL * H], f32)
            nc.sync.dma_start(out=off_sb, in_=off_t[:])
            zero_sb = consts.tile([1, HE], f32)
            nc.vector.memset(zero_sb, 0.0)
            c48_sb = consts.tile([128, 1], f32)
            nc.vector.memset(c48_sb, float(S1))

            tall_sb = consts.tile([96, H * 48], f32)
            mts_sb = consts.tile([128, BL * H * 48], f32)

            v_ap = v_t[:]
            pbv_ap = pbv_t[:]
            zpb_ap = zpb_t[:]

            sums_tiles = [None] * BL
            r2_tiles = [None] * BL
            zs_tiles = [None] * BL

            def emit_A(b):
                # reduce v -> rowsum (rows 0:48) | colsum (rows 64:112)
                ps1a = ps_acc.tile([128, HE], f32, tag="acc")
                for c in range(NCHUNK):
                    rhs_t = vt_pool.tile([128, HE], st1, tag="vt")
                    nc.gpsimd.dma_start(
                        out=rhs_t,
                        in_=v_ap[b, 1 + c * 128 : 1 + (c + 1) * 128, :],
                    )
                    at_sl = at_sb[:, bass.ts(c, 112)]
                    for half in range(2):
                        cols = slice(half * 512, (half + 1) * 512)
                        mm(
                            ps1a[0:112, cols], at_sl, rhs_t[:, cols],
                            start=(c == 0), stop=(c == NCHUNK - 1),
                        )
                sums_sb = sums_pool.tile([128, HE], f32, tag="sums")
                nc.vector.tensor_copy(out=sums_sb[0:48, :], in_=ps1a[0:48, :])
                nc.vector.tensor_copy(out=sums_sb[64:112, :], in_=ps1a[64:112, :])
                sums_tiles[b] = sums_sb

            def emit_phase0():
                # per-(b,h) matrices Mt = (exp(-off)*Mbase).T
                ez_sb = consts.tile([NPAD, H], f32)
                nc.scalar.activation(
                    out=ez_sb, in_=zsym_sb, func=mybir.ActivationFunctionType.Exp
                )
                esc_sb = consts.tile([1, BL * H], f32)
                nc.scalar.activation(
                    out=esc_sb, in_=off_sb,
                    func=mybir.ActivationFunctionType.Exp, scale=-1.0,
                )
                ps_eb = ps_small.tile([128, BL * H], f32, tag="sm")
                mm(ps_eb[0:112, :], ones_sb, esc_sb)
                escb_sb = consts.tile([128, BL * H], f32)
                nc.vector.tensor_copy(out=escb_sb[0:112, :], in_=ps_eb[0:112, :])

                ps_zre = ps_small.tile([96, H], f32, tag="sm")
                mm(ps_zre, frre_sb, ez_sb)
                zre_sb = consts.tile([96, H], f32)
                nc.vector.tensor_copy(out=zre_sb, in_=ps_zre)
                ps_zim = ps_small.tile([96, H], f32, tag="sm")
                mm(ps_zim, frim_sb, ez_sb)
                zim_sb = consts.tile([96, H], f32)
                nc.vector.tensor_copy(out=zim_sb, in_=ps_zim)

                for h in range(H):
                    sl = bass.ts(h, 48)
                    tmp_sb = small.tile([96, 48], f32, tag="ttmp")
                    nc.vector.tensor_scalar_mul(
                        out=tall_sb[:, sl], in0=a2_sb, scalar1=zre_sb[:, h : h + 1]
                    )
                    nc.vector.tensor_scalar_mul(
                        out=tmp_sb, in0=b2_sb, scalar1=zim_sb[:, h : h + 1]
                    )
                    nc.vector.tensor_add(tall_sb[:, sl], tall_sb[:, sl], tmp_sb)

                    # Mt_h = T_h.T @ G.T (i.e. Mbase.T), rows 0:48 and 64:112
                    ps_mt = ps_small.tile([128, 48], f32, tag="sm")
                    mm(ps_mt[0:48, :], tall_sb[:, sl], gt_sb)
                    mm(
                        ps_mt[64:112, :], tall_sb[:, sl], gt_sb,
                        tile_position=(0, 64), skip_group_check=True,
                    )
                    for b in range(BL):
                        bh = b * H + h
                        msl = bass.ts(bh, 48)
                        nc.vector.tensor_scalar_mul(
                            out=mts_sb[0:48, msl], in0=ps_mt[0:48, :],
                            scalar1=escb_sb[0:48, bh : bh + 1],
                        )
                        nc.vector.tensor_scalar_mul(
                            out=mts_sb[64:112, msl], in0=ps_mt[64:112, :],
                            scalar1=escb_sb[64:112, bh : bh + 1],
                        )

            def emit_edges(b):
                nc.sync.dma_start(out=pbv_ap[b, 0:1, :], in_=zero_sb[0:1, :])
                nc.sync.dma_start(
                    out=pbv_ap[b, SP2 - 1 : SP2, :], in_=zero_sb[0:1, :]
                )
                nc.sync.dma_start(out=zpb_ap[b, 0:1, :], in_=zero_sb[0:1, 0:H])
                nc.sync.dma_start(
                    out=zpb_ap[b, SP2 - 1 : SP2, :], in_=zero_sb[0:1, 0:H]
                )

            def emit_B(b):
                # RxU/RxV = Mt.T-matvec, plus zv columns
                sums_sb = sums_tiles[b]
                ps1b = ps_acc.tile([128, HE], f32, tag="acc")
                ps_z = ps_small.tile([128, H], f32, tag="sm")
                for h in range(H):
                    bh = b * H + h
                    msl = bass.ts(bh, 48)
                    hsl = bass.ts(h, 64)
                    mm(ps1b[0:48, hsl], mts_sb[0:48, msl], sums_sb[0:48, hsl])
                    mm(
                        ps1b[64:112, hsl], mts_sb[64:112, msl],
                        sums_sb[64:112, hsl], tile_position=(64, 64),
                        skip_group_check=True,
                    )
                    mm(ps_z[0:48, h : h + 1], mts_sb[0:48, msl], c48_sb[0:48, :])
                    mm(
                        ps_z[64:112, h : h + 1], mts_sb[64:112, msl],
                        c48_sb[64:112, :], tile_position=(64, 64),
                        skip_group_check=True,
                    )
                r2_sb = sums_pool.tile([128, HE], st2, tag="r2")
                nc.vector.memset(r2_sb, 0.0)
                nc.vector.tensor_copy(out=r2_sb[0:48, :], in_=ps1b[0:48, :])
                nc.vector.tensor_copy(out=r2_sb[64:112, :], in_=ps1b[64:112, :])
                zs_sb = sums_pool.tile([128, H], st2, tag="zs")
                nc.vector.memset(zs_sb, 0.0)
                nc.vector.tensor_copy(out=zs_sb[0:48, :], in_=ps_z[0:48, :])
                nc.vector.tensor_copy(out=zs_sb[64:112, :], in_=ps_z[64:112, :])
                r2_tiles[b] = r2_sb
                zs_tiles[b] = zs_sb

            def emit_C(b):
                # expand to [2304, 1024] output + z_pb
                r2_sb = r2_tiles[b]
                zs_sb = zs_tiles[b]
                for c in range(NCHUNK):
                    ew_sl = ew_sb[:, bass.ts(c, 128)]
                    ob = ob_pool.tile([128, HE], f32, tag="ob")
                    for half in range(2):
                        cols = slice(half * 512, (half + 1) * 512)
                        po = ps_out.tile([128, 512], f32, tag="po")
                        mm(po, ew_sl, r2_sb[:, cols])
                        nc.vector.tensor_copy(out=ob[:, cols], in_=po)
                    nc.scalar.dma_start(
                        out=pbv_ap[b, 1 + c * 128 : 1 + (c + 1) * 128, :], in_=ob
                    )
                    ps_zo = ps_small.tile([128, H], f32, tag="sm")
                    mm(ps_zo, ew_sl, zs_sb)
                    zb = ob_pool.tile([128, H], f32, tag="zb")
                    nc.scalar.copy(out=zb, in_=ps_zo)
                    nc.sync.dma_start(
                        out=zpb_ap[b, 1 + c * 128 : 1 + (c + 1) * 128, :], in_=zb
                    )

            def emit_warmup():
                # ~40 back-to-back tiny matmuls to lift the PE HAM clock gate
                # out of its cold 4/8 state before phase A's real work arrives.
                wsrc = consts.tile([128, 16], f32)
                nc.vector.memset(wsrc, 1.0)
                ps_w = ps_small.tile([128, 16], f32, tag="sm")
                for _ in range(40):
                    mm(ps_w[0:16, :], wsrc, wsrc)

            # emission order = scheduling priority: keep HBM fed end-to-end
            emit_warmup()
            emit_A(0)
            emit_phase0()
            emit_edges(0)
            emit_edges(1)
            emit_B(0)
            emit_A(1)
            emit_C(0)
            emit_B(1)
            emit_C(1)

    nc.compile()
    return nc


_NC_CACHE = {}


def kernel(v, offset, w):
    global LAST_RESULTS
    key = (ST1, ST2)
    if key not in _NC_CACHE:
        _NC_CACHE[key] = (_build_nc(), _build_host_constants())
    nc, hc = _NC_CACHE[key]

    v = np.ascontiguousarray(np.asarray(v, dtype=np.float32))
    offset = np.ascontiguousarray(np.asarray(offset, dtype=np.float32))
    w = np.ascontiguousarray(np.asarray(w, dtype=np.float32))

    zsymt = np.ascontiguousarray(w[0][:, hc["perm"]].T.astype(np.float32))  # [95, H]

    shared = {
        "zsymt": zsymt,
        "at_c": np.ascontiguousarray(hc["at"]),
        "ew_c": np.ascontiguousarray(hc["ew"]),
        "frre": hc["frre"], "frim": hc["frim"],
        "a2": hc["a2"], "b2": hc["b2"], "gt": hc["gt"],
        "ones112": hc["ones112"],
    }
    in_maps = []
    for i in range(NCORES):
        bsl = slice(i * BL, (i + 1) * BL)
        in_maps.append(
            {
                "v": np.ascontiguousarray(v[bsl].reshape(BL, SP2, HE)),
                "offset": np.ascontiguousarray(offset[bsl].reshape(1, BL * H)),
                **shared,
            }
        )

    res = run_bass_kernel_spmd(nc, in_maps, list(range(NCORES)), trace=TRACE)
    LAST_RESULTS = res

    pbv = np.concatenate(
        [res.results[i]["pbv"].reshape(BL, SP2, H, E) for i in range(NCORES)], axis=0
    )
    zpb = np.concatenate(
        [res.results[i]["zpb"].reshape(BL, SP2, H) for i in range(NCORES)], axis=0
    )
    return pbv.astype(np.float32), zpb.astype(np.float32)


# revision 10
# speedup vs baseline: 1.1751x; 1.1751x over previous
"""Trainium2 Bass kernel for nn_FFTBias2d (B=16, H=16, E=64, s=48).

Math: the reference's FFT pipeline collapses exactly. With
  z = exp(zsym - offset) = exp(-offset[b,h]) * exp(zsym[h]),
the per-(b,h) frequency response factors into a scalar exp(-offset) times a
per-head constant, so
  RxV = scale[b,h] * Mbase[h] @ colsum,   RxU = scale[b,h] * Mbase[h] @ rowsum,
  Mbase[h] = G @ diag_c(rfft95(exp(zsym[h]))) @ A48   (a real 48x48 matrix)
where G = (irfft94(.)[:48]) as a real matrix and A48 = rfft95 of a signal
supported on positions 47..94. rowsum/colsum are the 48-row/col sums of each
48x48 spatial image in v, and the output is the broadcast sum
  pbv[i*48+j] = RxU[i] + RxV[j],  z_pb analog with zv = 48*scale*Mbase@1.

Device pipeline per core (2 batches of the 16, data-parallel over 8 cores):
  phase 0: build Mt[b,h] = (scale*Mbase[h]).T on-chip from zsym/offset inputs
  phase A: 18 accumulating matmuls vs a 0/1 matrix -> rowsum||colsum  [reduce]
  phase B: per-head 48x48 matvecs -> RxU/RxV (+ zv columns)
  phase C: 18 expand matmuls vs a 0/1 matrix -> output tiles -> DMA   [expand]

Engine layout: loads are f32->f16 casting DMAs issued by GpSimd (SWDGE),
stores go out on the ACT HWDGE ring, PSUM evacuation on DVE. Emission order
interleaves batches (A0, ph0, B0, A1, C0, B1, C1) so HBM never idles.
"""

import numpy as np

import concourse.bass as bass
import concourse.mybir as mybir
from concourse import bacc
import concourse.tile as tile
from concourse.bass_utils import run_bass_kernel_spmd

B, SP2, H, E = 16, 2306, 16, 64
S1 = 48                      # spatial side
NPAD = 2 * S1 - 1            # 95
NIRFFT = 2 * (S1 - 1)        # 94
HE = H * E                   # 1024
ROWS = S1 * S1               # 2304
NCHUNK = ROWS // 128         # 18
NCORES = 8
BL = B // NCORES             # 2 batches per core

# dtype knobs for the two big matmul stages.
ST1 = "f16"   # reduce stage (v -> row/col sums)
ST2 = "f16"   # expand stage (RxU/RxV -> output)

TRACE = False
LAST_RESULTS = None

_DT = {"f32": mybir.dt.float32, "f16": mybir.dt.float16, "f32r": mybir.dt.float32}
_NPDT = {"f32": np.float32, "f16": np.float16, "f32r": np.float32}


def _build_host_constants():
    s = S1
    # G: irfft_94(.)[:48] as a real [48, 96] matrix acting on (Re, Im) stacked
    G = np.zeros((s, 2 * s), dtype=np.float64)
    for k in range(s):
        e = np.zeros(s, dtype=np.complex128)
        e[k] = 1.0
        G[:, k] = np.fft.irfft(e, n=NIRFFT)[:s]
        e[k] = 1j
        G[:, s + k] = np.fft.irfft(e, n=NIRFFT)[:s]

    # A48: rfft_95 of impulse at position 47+t  -> [48 bins, 48 positions]
    Are = np.zeros((s, s), dtype=np.float64)
    Aim = np.zeros((s, s), dtype=np.float64)
    for t in range(s):
        e = np.zeros(NPAD, dtype=np.float64)
        e[s - 1 + t] = 1.0
        F = np.fft.rfft(e)
        Are[:, t] = F.real
        Aim[:, t] = F.imag

    # rfft_95 full matrix -> [48, 95]
    FrRe = np.zeros((s, NPAD), dtype=np.float64)
    FrIm = np.zeros((s, NPAD), dtype=np.float64)
    for p in range(NPAD):
        e = np.zeros(NPAD, dtype=np.float64)
        e[p] = 1.0
        F = np.fft.rfft(e)
        FrRe[:, p] = F.real
        FrIm[:, p] = F.imag

    # duplicated-rfft weights: out rows 0:48 and 48:96 both get the same part
    frre_dup = np.zeros((NPAD, 96), dtype=np.float32)
    frre_dup[:, 0:48] = FrRe.T
    frre_dup[:, 48:96] = FrRe.T
    frim_dup = np.zeros((NPAD, 96), dtype=np.float32)
    frim_dup[:, 0:48] = FrIm.T
    frim_dup[:, 48:96] = FrIm.T

    # a2/b2: T = a2 * Zre_dup + b2 * Zim_dup  gives [re_rows; im_rows] of
    # diag_c(Z) @ A48:  re = Zre*Are - Zim*Aim ; im = Zre*Aim + Zim*Are
    a2 = np.zeros((96, 48), dtype=np.float32)
    a2[0:48] = Are
    a2[48:96] = Aim
    b2 = np.zeros((96, 48), dtype=np.float32)
    b2[0:48] = -Aim
    b2[48:96] = Are

    gt = np.ascontiguousarray(G.T.astype(np.float32))  # [96, 48]

    # reduce-stage weight: lhsT [128, 112] per chunk;
    # cols 0:48 pick rowsum (i = p//48), cols 64:112 pick colsum (j = p%48)
    at = np.zeros((128, NCHUNK * 112), dtype=np.float64)
    for c in range(NCHUNK):
        for k in range(128):
            p = c * 128 + k
            at[k, c * 112 + (p // S1)] = 1.0
            at[k, c * 112 + 64 + (p % S1)] = 1.0

    # expand-stage weight: lhsT [128, 128] per chunk;
    # out row m (global p = c*128+m) = rhs[i(p)] + rhs[64 + j(p)]
    ew = np.zeros((128, NCHUNK * 128), dtype=np.float64)
    for c in range(NCHUNK):
        for m in range(128):
            p = c * 128 + m
            ew[p // S1, c * 128 + m] = 1.0
            ew[64 + (p % S1), c * 128 + m] = 1.0

    ones112 = np.ones((1, 112), dtype=np.float32)

    perm = np.concatenate(
        [[S1 - 1], np.arange(S1 - 1, 0, -1), np.arange(0, S1 - 1)]
    )  # z = concat(w[-1:], w[1:][::-1], w[:-1])

    return {
        "frre": frre_dup, "frim": frim_dup, "a2": a2, "b2": b2, "gt": gt,
        "at": at.astype(_NPDT[ST1]), "ew": ew.astype(_NPDT[ST2]),
        "ones112": ones112, "perm": perm,
    }


def _build_nc():
    f32 = mybir.dt.float32
    st1 = _DT[ST1]
    st2 = _DT[ST2]

    nc = bacc.Bacc()

    v_t = nc.declare_dram_parameter("v", [BL, SP2, HE], f32, isOutput=False)
    off_t = nc.declare_dram_parameter("offset", [1, BL * H], f32, isOutput=False)
    zsym_t = nc.declare_dram_parameter("zsymt", [NPAD, H], f32, isOutput=False)
    at_t = nc.declare_dram_parameter("at_c", [128, NCHUNK * 112], st1, isOutput=False)
    ew_t = nc.declare_dram_parameter("ew_c", [128, NCHUNK * 128], st2, isOutput=False)
    frre_t = nc.declare_dram_parameter("frre", [NPAD, 96], f32, isOutput=False)
    frim_t = nc.declare_dram_parameter("frim", [NPAD, 96], f32, isOutput=False)
    a2_t = nc.declare_dram_parameter("a2", [96, 48], f32, isOutput=False)
    b2_t = nc.declare_dram_parameter("b2", [96, 48], f32, isOutput=False)
    gt_t = nc.declare_dram_parameter("gt", [96, 48], f32, isOutput=False)
    ones_t = nc.declare_dram_parameter("ones112", [1, 112], f32, isOutput=False)

    pbv_t = nc.declare_dram_parameter("pbv", [BL, SP2, HE], f32, isOutput=True)
    zpb_t = nc.declare_dram_parameter("zpb", [BL, SP2, H], f32, isOutput=True)

    def mm(out, lhsT, rhs, **kw):
        nc.tensor.matmul(out, lhsT, rhs, **kw)

    with tile.TileContext(nc) as tc:
        from contextlib import ExitStack

        with ExitStack() as ctx:
            consts = ctx.enter_context(tc.tile_pool(name="consts", bufs=1))
            small = ctx.enter_context(tc.tile_pool(name="small", bufs=2))
            vt_pool = ctx.enter_context(tc.tile_pool(name="vt", bufs=8))
            sums_pool = ctx.enter_context(tc.tile_pool(name="sums", bufs=2))
            ob_pool = ctx.enter_context(tc.tile_pool(name="ob", bufs=3))
            ps_acc = ctx.enter_context(tc.tile_pool(name="ps_acc", bufs=2, space="PSUM"))
            ps_out = ctx.enter_context(tc.tile_pool(name="ps_out", bufs=2, space="PSUM"))
            ps_small = ctx.enter_context(tc.tile_pool(name="ps_small", bufs=2, space="PSUM"))

            # ---- constants ----
            at_sb = consts.tile([128, NCHUNK * 112], st1)
            nc.sync.dma_start(out=at_sb, in_=at_t[:])
            ew_sb = consts.tile([128, NCHUNK * 128], st2)
            nc.sync.dma_start(out=ew_sb, in_=ew_t[:])
            frre_sb = consts.tile([NPAD, 96], f32)
            nc.sync.dma_start(out=frre_sb, in_=frre_t[:])
            frim_sb = consts.tile([NPAD, 96], f32)
            nc.sync.dma_start(out=frim_sb, in_=frim_t[:])
            a2_sb = consts.tile([96, 48], f32)
            nc.sync.dma_start(out=a2_sb, in_=a2_t[:])
            b2_sb = consts.tile([96, 48], f32)
            nc.sync.dma_start(out=b2_sb, in_=b2_t[:])
            gt_sb = consts.tile([96, 48], f32)
            nc.sync.dma_start(out=gt_sb, in_=gt_t[:])
            ones_sb = consts.tile([1, 112], f32)
            nc.sync.dma_start(out=ones_sb, in_=ones_t[:])
            zsym_sb = consts.tile([NPAD, H], f32)
            nc.sync.dma_start(out=zsym_sb, in_=zsym_t[:])
            off_sb = consts.tile([1, BL * H], f32)
            nc.sync.dma_start(out=off_sb, in_=off_t[:])
            zero_sb = consts.tile([1, HE], f32)
            nc.vector.memset(zero_sb, 0.0)
            c48_sb = consts.tile([128, 1], f32)
            nc.vector.memset(c48_sb, float(S1))

            tall_sb = consts.tile([96, H * 48], f32)
            mts_sb = consts.tile([128, BL * H * 48], f32)

            v_ap = v_t[:]
            pbv_ap = pbv_t[:]
            zpb_ap = zpb_t[:]

            sums_tiles = [None] * BL
            r2_tiles = [None] * BL
            zs_tiles = [None] * BL

            def emit_A(b):
                # reduce v -> rowsum (rows 0:48) | colsum (rows 64:112)
                ps1a = ps_acc.tile([128, HE], f32, tag="acc")
                for c in range(NCHUNK):
                    rhs_t = vt_pool.tile([128, HE], st1, tag="vt")
                    nc.gpsimd.dma_start(
                        out=rhs_t,
                        in_=v_ap[b, 1 + c * 128 : 1 + (c + 1) * 128, :],
                    )
                    at_sl = at_sb[:, bass.ts(c, 112)]
                    for half in range(2):
                        cols = slice(half * 512, (half + 1) * 512)
                        mm(
                            ps1a[0:112, cols], at_sl, rhs_t[:, cols],
                            start=(c == 0), stop=(c == NCHUNK - 1),
                        )
                sums_sb = sums_pool.tile([128, HE], f32, tag="sums")
                nc.vector.tensor_copy(out=sums_sb[0:48, :], in_=ps1a[0:48, :])
                nc.vector.tensor_copy(out=sums_sb[64:112, :], in_=ps1a[64:112, :])
                sums_tiles[b] = sums_sb

            def emit_phase0():
                # per-(b,h) matrices Mt = (exp(-off)*Mbase).T
                ez_sb = consts.tile([NPAD, H], f32)
                nc.scalar.activation(
                    out=ez_sb, in_=zsym_sb, func=mybir.ActivationFunctionType.Exp
                )
                esc_sb = consts.tile([1, BL * H], f32)
                nc.scalar.activation(
                    out=esc_sb, in_=off_sb,
                    func=mybir.ActivationFunctionType.Exp, scale=-1.0,
                )
                ps_eb = ps_small.tile([128, BL * H], f32, tag="sm")
                mm(ps_eb[0:112, :], ones_sb, esc_sb)
                escb_sb = consts.tile([128, BL * H], f32)
                nc.vector.tensor_copy(out=escb_sb[0:112, :], in_=ps_eb[0:112, :])

                ps_zre = ps_small.tile([96, H], f32, tag="sm")
                mm(ps_zre, frre_sb, ez_sb)
                zre_sb = consts.tile([96, H], f32)
                nc.vector.tensor_copy(out=zre_sb, in_=ps_zre)
                ps_zim = ps_small.tile([96, H], f32, tag="sm")
                mm(ps_zim, frim_sb, ez_sb)
                zim_sb = consts.tile([96, H], f32)
                nc.vector.tensor_copy(out=zim_sb, in_=ps_zim)

                for h in range(H):
                    sl = bass.ts(h, 48)
                    tmp_sb = small.tile([96, 48], f32, tag="ttmp")
                    nc.vector.tensor_scalar_mul(
                        out=tall_sb[:, sl], in0=a2_sb, scalar1=zre_sb[:, h : h + 1]
                    )
                    nc.vector.tensor_scalar_mul(
                        out=tmp_sb, in0=b2_sb, scalar1=zim_sb[:, h : h + 1]
                    )
                    nc.vector.tensor_add(tall_sb[:, sl], tall_sb[:, sl], tmp_sb)

                    # Mt_h = T_h.T @ G.T (i.e. Mbase.T), rows 0:48 and 64:112
                    ps_mt = ps_small.tile([128, 48], f32, tag="sm")
                    mm(ps_mt[0:48, :], tall_sb[:, sl], gt_sb)
                    mm(
                        ps_mt[64:112, :], tall_sb[:, sl], gt_sb,
                        tile_position=(0, 64), skip_group_check=True,
                    )
                    for b in range(BL):
                        bh = b * H + h
                        msl = bass.ts(bh, 48)
                        nc.vector.tensor_scalar_mul(
                            out=mts_sb[0:48, msl], in0=ps_mt[0:48, :],
                            scalar1=escb_sb[0:48, bh : bh + 1],
                        )
                        nc.vector.tensor_scalar_mul(
                            out=mts_sb[64:112, msl], in0=ps_mt[64:112, :],
                            scalar1=escb_sb[64:112, bh : bh + 1],
                        )

            def emit_edges(b):
                nc.sync.dma_start(out=pbv_ap[b, 0:1, :], in_=zero_sb[0:1, :])
                nc.sync.dma_start(
                    out=pbv_ap[b, SP2 - 1 : SP2, :], in_=zero_sb[0:1, :]
                )
                nc.sync.dma_start(out=zpb_ap[b, 0:1, :], in_=zero_sb[0:1, 0:H])
                nc.sync.dma_start(
                    out=zpb_ap[b, SP2 - 1 : SP2, :], in_=zero_sb[0:1, 0:H]
                )

            def emit_B(b):
                # RxU/RxV = Mt.T-matvec, plus zv columns
                sums_sb = sums_tiles[b]
                ps1b = ps_acc.tile([128, HE], f32, tag="acc")
                ps_z = ps_small.tile([128, H], f32, tag="sm")
                for h in range(H):
                    bh = b * H + h
                    msl = bass.ts(bh, 48)
                    hsl = bass.ts(h, 64)
                    mm(ps1b[0:48, hsl], mts_sb[0:48, msl], sums_sb[0:48, hsl])
                    mm(
                        ps1b[64:112, hsl], mts_sb[64:112, msl],
                        sums_sb[64:112, hsl], tile_position=(64, 64),
                        skip_group_check=True,
                    )
                    mm(ps_z[0:48, h : h + 1], mts_sb[0:48, msl], c48_sb[0:48, :])
                    mm(
                        ps_z[64:112, h : h + 1], mts_sb[64:112, msl],
                        c48_sb[64:112, :], tile_position=(64, 64),
                        skip_group_check=True,
                    )
                r2_sb = sums_pool.tile([128, HE], st2, tag="r2")
                nc.vector.memset(r2_sb, 0.0)
                nc.vector.tensor_copy(out=r2_sb[0:48, :], in_=ps1b[0:48, :])
                nc.vector.tensor_copy(out=r2_sb[64:112, :], in_=ps1b[64:112, :])
                zs_sb = sums_pool.tile([128, H], st2, tag="zs")
                nc.vector.memset(zs_sb, 0.0)
                nc.vector.tensor_copy(out=zs_sb[0:48, :], in_=ps_z[0:48, :])
                nc.vector.tensor_copy(out=zs_sb[64:112, :], in_=ps_z[64:112, :])
                r2_tiles[b] = r2_sb
                zs_tiles[b] = zs_sb

            def emit_C(b):
                # expand to [2304, 1024] output + z_pb
                r2_sb = r2_tiles[b]
                zs_sb = zs_tiles[b]
                for c in range(NCHUNK):
                    ew_sl = ew_sb[:, bass.ts(c, 128)]
                    ob = ob_pool.tile([128, HE], f32, tag="ob")
                    for half in range(2):
                        cols = slice(half * 512, (half + 1) * 512)
                        po = ps_out.tile([128, 512], f32, tag="po")
                        mm(po, ew_sl, r2_sb[:, cols])
                        nc.vector.tensor_copy(out=ob[:, cols], in_=po)
                    nc.scalar.dma_start(
                        out=pbv_ap[b, 1 + c * 128 : 1 + (c + 1) * 128, :], in_=ob
                    )
                    ps_zo = ps_small.tile([128, H], f32, tag="sm")
                    mm(ps_zo, ew_sl, zs_sb)
                    zb = ob_pool.tile([128, H], f32, tag="zb")
                    nc.scalar.copy(out=zb, in_=ps_zo)
                    nc.sync.dma_start(
                        out=zpb_ap[b, 1 + c * 128 : 1 + (c + 1) * 128, :], in_=zb
                    )

            def emit_warmup():
                # ~40 back-to-back tiny matmuls to lift the PE HAM clock gate
                # out of its cold 4/8 state before phase A's real work arrives.
                wsrc = consts.tile([128, 16], f32)
                nc.vector.memset(wsrc, 1.0)
                ps_w = ps_small.tile([128, 16], f32, tag="sm")
                for _ in range(40):
                    mm(ps_w[0:16, :], wsrc, wsrc)

            # emission order = scheduling priority: keep HBM fed end-to-end
            emit_warmup()
            emit_A(0)
            emit_phase0()
            emit_edges(0)
            emit_edges(1)
            emit_B(0)
            emit_A(1)
            emit_C(0)
            emit_B(1)
            emit_C(1)

    nc.compile()
    return nc


_NC_CACHE = {}


def kernel(v, offset, w):
    global LAST_RESULTS
    key = (ST1, ST2)
    if key not in _NC_CACHE:
        _NC_CACHE[key] = (_build_nc(), _build_host_constants())
    nc, hc = _NC_CACHE[key]

    v = np.ascontiguousarray(np.asarray(v, dtype=np.float32))
    offset = np.ascontiguousarray(np.asarray(offset, dtype=np.float32))
    w = np.ascontiguousarray(np.asarray(w, dtype=np.float32))

    zsymt = np.ascontiguousarray(w[0][:, hc["perm"]].T.astype(np.float32))  # [95, H]

    shared = {
        "zsymt": zsymt,
        "at_c": np.ascontiguousarray(hc["at"]),
        "ew_c": np.ascontiguousarray(hc["ew"]),
        "frre": hc["frre"], "frim": hc["frim"],
        "a2": hc["a2"], "b2": hc["b2"], "gt": hc["gt"],
        "ones112": hc["ones112"],
    }
    in_maps = []
    for i in range(NCORES):
        bsl = slice(i * BL, (i + 1) * BL)
        in_maps.append(
            {
                "v": np.ascontiguousarray(v[bsl].reshape(BL, SP2, HE)),
                "offset": np.ascontiguousarray(offset[bsl].reshape(1, BL * H)),
                **shared,
            }
        )

    res = run_bass_kernel_spmd(nc, in_maps, list(range(NCORES)), trace=TRACE)
    LAST_RESULTS = res

    pbv = np.concatenate(
        [res.results[i]["pbv"].reshape(BL, SP2, H, E) for i in range(NCORES)], axis=0
    )
    zpb = np.concatenate(
        [res.results[i]["zpb"].reshape(BL, SP2, H) for i in range(NCORES)], axis=0
    )
    return pbv.astype(np.float32), zpb.astype(np.float32)
